# revision 9
# baseline (speedup 1.0000x reference)
"""Trainium2 Bass kernel for nn_DifferentiableDenseHGTConv.

Self-contained: takes FULL inputs as numpy arrays, shards batch x target-row
halves across 8 NeuronCores (core c -> batch c//2, row-half c%2), runs one
SPMD Bass/Tile kernel, gathers the full [4,1024,256] fp32 output.

Per-core dataflow (I = 512 target rows, N = 1024 sources, H=8 heads, DK=32):
  All [N,N,H]-scale tensors live in [j(source)-partition, i(target)-free]
  orientation so the attention-weighted aggregation matmuls contract over j
  in PE partitions.  Logits res_att^T = sum_r m_r (.) (K~bar x Q~_r) are
  built by PE matmuls (bf16) + DVE masked sums; softmax denominators and the
  row-sum gate are partition reductions done as PE ones-matmuls; the
  normalization (cond/Z) is folded to the aggregation epilogue.  The
  relation/type structure is factorized so rel_att (+pri/sqrt(dk)) is folded
  into per-relation queries Q~_r and rel_msg is applied once at the end on
  the [I,256] aggregate (block-diagonal 256x256 matmuls).
"""
import math
import numpy as np
import ml_dtypes

T, R, H, DK = 3, 4, 8, 32
OUT = H * DK          # 256
IN = 256
B, N = 4, 1024
I = 512               # target rows per core
NCORES = 8
LN_EPS = 1e-5
BF16 = ml_dtypes.bfloat16

_built = {}


def _build_nc():
    """Build + compile the SPMD Bass module once per process."""
    if "nc" in _built:
        return _built["nc"]

    from contextlib import ExitStack
    import concourse.bass as bass
    import concourse.tile as tile
    from concourse import bacc, mybir
    from concourse.masks import make_identity

    dt = mybir.dt
    AF = mybir.ActivationFunctionType
    ALU = mybir.AluOpType

    nc = bacc.Bacc("TRN2", target_bir_lowering=False, debug=False,
                   enable_asserts=True, num_devices=NCORES)

    # ---------------- DRAM parameters (per core) ----------------
    xT = nc.declare_dram_parameter("xT", [IN + 1, N], dt.bfloat16, isOutput=False)
    xTI = nc.declare_dram_parameter("xTI", [IN + 1, I], dt.bfloat16, isOutput=False)
    xI = nc.declare_dram_parameter("xI", [I, IN], dt.bfloat16, isOutput=False)
    xTs = nc.declare_dram_parameter("xTs", [T, IN + 1, N], dt.bfloat16, isOutput=False)
    xTsI = nc.declare_dram_parameter("xTsI", [T, IN + 1, I], dt.bfloat16, isOutput=False)
    typesI = nc.declare_dram_parameter("typesI", [I, T], dt.float32, isOutput=False)
    adjT = nc.declare_dram_parameter("adjT", [N, I], dt.bfloat16, isOutput=False)
    erelT = nc.declare_dram_parameter("erelT", [R, N, I], dt.bfloat16, isOutput=False)
    wq = nc.declare_dram_parameter("wq", [T, IN + 1, OUT], dt.bfloat16, isOutput=False)
    wk = nc.declare_dram_parameter("wk", [T, IN + 1, OUT], dt.bfloat16, isOutput=False)
    wv = nc.declare_dram_parameter("wv", [T, IN + 1, OUT], dt.bfloat16, isOutput=False)
    abd = nc.declare_dram_parameter("abd", [R, OUT, OUT], dt.bfloat16, isOutput=False)
    bbd = nc.declare_dram_parameter("bbd", [R, OUT, OUT], dt.bfloat16, isOutput=False)
    wa = nc.declare_dram_parameter("wa", [T, OUT + 1, OUT], dt.bfloat16, isOutput=False)
    lng = nc.declare_dram_parameter("lng", [T, OUT], dt.bfloat16, isOutput=False)
    lnb = nc.declare_dram_parameter("lnb", [T, OUT], dt.bfloat16, isOutput=False)
    alph = nc.declare_dram_parameter("alph", [2, T], dt.float32, isOutput=False)
    outP = nc.declare_dram_parameter("out", [I, OUT], dt.float32, isOutput=True)

    def bcast(row_ap, parts=128):
        # DMA access pattern replicating one DRAM row across `parts` partitions
        return bass.AP(tensor=row_ap.tensor, offset=row_ap.offset,
                       ap=[[0, parts]] + list(row_ap.ap[1:]))

    NJC = N // 128    # 8 source chunks
    NIB = I // 128    # 4 target blocks
    OC = OUT // 128   # 2 output-dim chunks

    with ExitStack() as ctx:
        tc = ctx.enter_context(tile.TileContext(nc))
        const = ctx.enter_context(tc.tile_pool(name="const", bufs=1))
        work = ctx.enter_context(tc.tile_pool(name="work", bufs=3))

        # ---------------- persistent constants ----------------
        ident = const.tile([128, 128], dt.bfloat16, tag="ident", name="ident")
        make_identity(nc, ident)
        onesCol = const.tile([128, 1], dt.bfloat16, tag="onescol", name="onescol")
        nc.vector.memset(onesCol, 1.0)
        onesRow = const.tile([1, I], dt.bfloat16, tag="onesrow", name="onesrow")
        nc.vector.memset(onesRow, 1.0)

        xI_t = []
        for ib in range(NIB):
            tl = const.tile([128, IN], dt.bfloat16, tag=f"xI{ib}", name=f"xI{ib}")
            nc.sync.dma_start(out=tl, in_=xI[ib * 128:(ib + 1) * 128, :])
            xI_t.append(tl)
        tyI_t = []
        for ib in range(NIB):
            tl = const.tile([128, T], dt.float32, tag=f"tyI{ib}", name=f"tyI{ib}")
            nc.sync.dma_start(out=tl, in_=typesI[ib * 128:(ib + 1) * 128, :])
            tyI_t.append(tl)
        gB, bB = [], []
        for t_ in range(T):
            tl = const.tile([128, OUT], dt.bfloat16, tag=f"gB{t_}", name=f"gB{t_}")
            nc.gpsimd.dma_start(out=tl, in_=bcast(lng[t_:t_ + 1, :]))
            gB.append(tl)
            tl = const.tile([128, OUT], dt.bfloat16, tag=f"bB{t_}", name=f"bB{t_}")
            nc.gpsimd.dma_start(out=tl, in_=bcast(lnb[t_:t_ + 1, :]))
            bB.append(tl)
        alphB, alph1mB = [], []
        for t_ in range(T):
            tl = const.tile([128, 1], dt.float32, tag=f"alB{t_}", name=f"alB{t_}")
            nc.gpsimd.dma_start(out=tl, in_=bcast(alph[0:1, t_:t_ + 1]))
            alphB.append(tl)
            tl = const.tile([128, 1], dt.float32, tag=f"al1B{t_}", name=f"al1B{t_}")
            nc.gpsimd.dma_start(out=tl, in_=bcast(alph[1:2, t_:t_ + 1]))
            alph1mB.append(tl)

        def load_w3(pool, param, name, klen=IN):
            # [klen+1, OUT] augmented weight -> [128,256] x(klen//128) + [1,256]
            out = []
            for t_ in range(T):
                tls = []
                for kc in range(klen // 128):
                    tl = pool.tile([128, OUT], dt.bfloat16, tag=f"{name}{t_}_{kc}",
                                   name=f"{name}{t_}_{kc}")
                    nc.sync.dma_start(out=tl, in_=param[t_, kc * 128:(kc + 1) * 128, :])
                    tls.append(tl)
                tl = pool.tile([1, OUT], dt.bfloat16, tag=f"{name}{t_}_a",
                               name=f"{name}{t_}_a")
                nc.sync.dma_start(out=tl, in_=param[t_, klen:klen + 1, :])
                tls.append(tl)
                out.append(tls)
            return out

        wa_t = load_w3(const, wa, "wa", klen=OUT)
        bbd_t = []
        for r in range(R):
            tls = []
            for kc in range(OC):
                tl = const.tile([128, OUT], dt.bfloat16, tag=f"bbd{r}_{kc}",
                                name=f"bbd{r}_{kc}")
                nc.sync.dma_start(out=tl, in_=bbd[r, kc * 128:(kc + 1) * 128, :])
                tls.append(tl)
            bbd_t.append(tls)

        # Head-sliced PE operands are packed 3 heads per tile (32-row bands at
        # base partitions 0/32/64 -- the only legal matmul base partitions).
        def hs(tiles, h, lo=None, hi=None):
            b = (h % 3) * 32
            if lo is None:
                return tiles[h // 3][b:b + 32, :]
            return tiles[h // 3][b:b + 32, lo:hi]

        def hpack_alloc(name_, width):
            return [const.tile([96, width], dt.bfloat16, tag=f"{name_}0", name=f"{name_}0"),
                    const.tile([96, width], dt.bfloat16, tag=f"{name_}1", name=f"{name_}1"),
                    const.tile([64, width], dt.bfloat16, tag=f"{name_}2", name=f"{name_}2")]

        # ---------------- stage A: projections ----------------
        # Host pre-scales x by each soft-type column (aug row = types row), so
        # K~_s^T / QbarT / KbarT are plain augmented matmuls with PSUM
        # accumulation doing the type mixing.
        ktilT = []                                # per s: 3 packed tiles [., N]
        kbarT = hpack_alloc("kbar", N)
        qtilT = []                                # per r: 3 packed tiles [., I]
        vt = [[None] * NJC for _ in range(T)]     # Vt_s [N, 256]
        with tc.tile_pool(name="sA", bufs=1) as sA:
            xT_t = []
            for kc in range(2):
                tl = sA.tile([128, N], dt.bfloat16, tag=f"xT{kc}", name=f"xT{kc}")
                nc.sync.dma_start(out=tl, in_=xT[kc * 128:(kc + 1) * 128, :])
                xT_t.append(tl)
            xT_t.append(sA.tile([1, N], dt.bfloat16, tag="xT2", name="xT2"))
            nc.sync.dma_start(out=xT_t[2], in_=xT[IN:IN + 1, :])
            wq_t = load_w3(sA, wq, "wq")
            wk_t = load_w3(sA, wk, "wk")
            wv_t = load_w3(sA, wv, "wv")
            abd_t = []
            for r in range(R):
                tls = []
                for kc in range(OC):
                    tl = sA.tile([128, OUT], dt.bfloat16, tag=f"abd{r}_{kc}",
                                 name=f"abd{r}_{kc}")
                    nc.sync.dma_start(out=tl, in_=abd[r, kc * 128:(kc + 1) * 128, :])
                    tls.append(tl)
                abd_t.append(tls)

            # K side: K~T_s and KbarT via PSUM accumulation over (s, kc)
            for s in range(T):
                ktilT.append(hpack_alloc(f"ktil{s}_", N))
            with tc.tile_pool(name="psK", bufs=2, space="PSUM") as psK, \
                 tc.tile_pool(name="psKb", bufs=1, space="PSUM") as psKb:
                kbarP = [psKb.tile([128, N], dt.float32, tag=f"kbarP{oc}",
                                   name=f"kbarP{oc}") for oc in range(OC)]
                for s in range(T):
                    xs_t = []
                    for kc in range(2):
                        tl = sA.tile([128, N], dt.bfloat16, tag=f"xs{kc}",
                                     name=f"xs{kc}", bufs=1)
                        nc.sync.dma_start(out=tl, in_=xTs[s, kc * 128:(kc + 1) * 128, :])
                        xs_t.append(tl)
                    tl = sA.tile([1, N], dt.bfloat16, tag="xs2", name="xs2", bufs=1)
                    nc.sync.dma_start(out=tl, in_=xTs[s, IN:IN + 1, :])
                    xs_t.append(tl)
                    for oc in range(OC):
                        pa = psK.tile([128, N], dt.float32, tag="pA", name="pA")
                        for kc in range(3):
                            for nh in range(2):
                                nsl = slice(nh * 512, (nh + 1) * 512)
                                nc.tensor.matmul(pa[:, nsl],
                                                 wk_t[s][kc][:, oc * 128:(oc + 1) * 128],
                                                 xs_t[kc][:, nsl],
                                                 start=(kc == 0), stop=(kc == 2))
                                nc.tensor.matmul(kbarP[oc][:, nsl],
                                                 wk_t[s][kc][:, oc * 128:(oc + 1) * 128],
                                                 xs_t[kc][:, nsl],
                                                 start=(s == 0 and kc == 0),
                                                 stop=(s == T - 1 and kc == 2))
                        kt = sA.tile([128, N], dt.bfloat16, tag="ktev", name="ktev",
                                     bufs=1)
                        nc.scalar.copy(kt, pa)
                        for hh in range(4):
                            h = oc * 4 + hh
                            nc.sync.dma_start(out=hs(ktilT[s], h),
                                              in_=kt[hh * 32:(hh + 1) * 32, :])
                for oc in range(OC):
                    kb = sA.tile([128, N], dt.bfloat16, tag="kbev", name="kbev",
                                 bufs=1)
                    nc.scalar.copy(kb, kbarP[oc])
                    for hh in range(4):
                        h = oc * 4 + hh
                        nc.sync.dma_start(out=hs(kbarT, h),
                                          in_=kb[hh * 32:(hh + 1) * 32, :])

            # Q side: QbarT via PSUM accumulation over (t, kc), then rel_att fold
            qbarT = []
            with tc.tile_pool(name="psQ", bufs=2, space="PSUM") as psQ:
                for oc in range(OC):
                    pa = psQ.tile([128, I], dt.float32, tag="pQb", name="pQb")
                    for t_ in range(T):
                        xsi_t = []
                        for kc in range(2):
                            tl = sA.tile([128, I], dt.bfloat16, tag=f"xsi{kc}",
                                         name=f"xsi{kc}", bufs=1)
                            nc.sync.dma_start(out=tl,
                                              in_=xTsI[t_, kc * 128:(kc + 1) * 128, :])
                            xsi_t.append(tl)
                        tl = sA.tile([1, I], dt.bfloat16, tag="xsi2", name="xsi2",
                                     bufs=1)
                        nc.sync.dma_start(out=tl, in_=xTsI[t_, IN:IN + 1, :])
                        xsi_t.append(tl)
                        for kc in range(3):
                            nc.tensor.matmul(pa,
                                             wq_t[t_][kc][:, oc * 128:(oc + 1) * 128],
                                             xsi_t[kc], start=(t_ == 0 and kc == 0),
                                             stop=(t_ == T - 1 and kc == 2))
                    qb = sA.tile([128, I], dt.bfloat16, tag=f"qbar{oc}",
                                 name=f"qbar{oc}")
                    nc.scalar.copy(qb, pa)
                    qbarT.append(qb)
                for r in range(R):
                    qtilT.append(hpack_alloc(f"qtil{r}_", I))
                    for ot in range(OC):
                        pa = psQ.tile([128, I], dt.float32, tag="pQb", name="pQb")
                        for kc in range(OC):
                            nc.tensor.matmul(pa,
                                             abd_t[r][kc][:, ot * 128:(ot + 1) * 128],
                                             qbarT[kc], start=(kc == 0), stop=(kc == 1))
                        qt = sA.tile([128, I], dt.bfloat16, tag="qt128", name="qt128",
                                     bufs=1)
                        nc.scalar.copy(qt, pa)
                        for hh in range(4):
                            h = ot * 4 + hh
                            nc.sync.dma_start(out=hs(qtilT[r], h),
                                              in_=qt[hh * 32:(hh + 1) * 32, :])

            # V side: Vt_s[j, o]
            with tc.tile_pool(name="psV", bufs=3, space="PSUM") as psV:
                for s in range(T):
                    for jb in range(NJC):
                        pa = psV.tile([128, OUT], dt.float32, tag="pV", name="pV")
                        for kc in range(3):
                            nc.tensor.matmul(pa,
                                             xT_t[kc][:, jb * 128:(jb + 1) * 128],
                                             wv_t[s][kc], start=(kc == 0), stop=(kc == 2))
                        v = const.tile([128, OUT], dt.bfloat16, tag=f"vt{s}_{jb}",
                                       name=f"vt{s}_{jb}")
                        nc.scalar.copy(v, pa)
                        vt[s][jb] = v

        # masks m_r^T[j, i] = adjT (.) erelT_r
        mT = [[None] * NJC for _ in range(R)]
        dma_q = [nc.sync, nc.scalar]
        for jc in range(NJC):
            at = work.tile([128, I], dt.bfloat16, tag="adj_in", name="adj_in", bufs=2)
            dma_q[jc % 2].dma_start(out=at, in_=adjT[jc * 128:(jc + 1) * 128, :])
            for r in range(R):
                et = work.tile([128, I], dt.bfloat16, tag="erel_in", name="erel_in",
                               bufs=4)
                dma_q[(jc + r) % 2].dma_start(out=et,
                                              in_=erelT[r, jc * 128:(jc + 1) * 128, :])
                m = const.tile([128, I], dt.bfloat16, tag=f"m{r}_{jc}",
                               name=f"m{r}_{jc}")
                nc.vector.tensor_mul(m, at, et)
                mT[r][jc] = m

        # (1-alpha_t) * x residual tiles
        xa = [[None] * NIB for _ in range(T)]
        for t_ in range(T):
            for ib in range(NIB):
                tl = const.tile([128, IN], dt.bfloat16, tag=f"xa{t_}_{ib}",
                                name=f"xa{t_}_{ib}")
                nc.vector.tensor_scalar(out=tl, in0=xI_t[ib],
                                        scalar1=alph1mB[t_], scalar2=None,
                                        op0=ALU.mult)
                xa[t_][ib] = tl

        # ---------------- main h-loop ----------------
        aggn = [[None] * NIB for _ in range(R)]  # agg'_r / Z, [I, (h,d')] bf16
        for r in range(R):
            for ib in range(NIB):
                aggn[r][ib] = const.tile([128, OUT], dt.bfloat16,
                                         tag=f"aggn{r}_{ib}", name=f"aggn{r}_{ib}")

        with tc.tile_pool(name="psM", bufs=2, space="PSUM") as psM, \
             tc.tile_pool(name="psG", bufs=1, space="PSUM") as psG, \
             tc.tile_pool(name="psZ", bufs=2, space="PSUM") as psZ, \
             tc.tile_pool(name="psR", bufs=1, space="PSUM") as psR, \
             tc.tile_pool(name="ph", bufs=10) as ph:

            rsP = psR.tile([128, NIB], dt.float32, tag="rs", name="rs")
            for h in range(H):
                zP = psZ.tile([128, NIB], dt.float32, tag="z", name="z")
                aggp = psG.tile([128, 512], dt.float32, tag="aggp", name="aggp")
                # ---- phase 1: logits (wide paired evacs), batched exp ----
                res_h = []
                for jc in range(NJC):
                    sbw = []
                    for rp in range(2):
                        ps = psM.tile([128, 2 * I], dt.float32, tag="sc", name="sc")
                        for rr in range(2):
                            nc.tensor.matmul(ps[:, rr * I:(rr + 1) * I],
                                             hs(kbarT, h, jc * 128, (jc + 1) * 128),
                                             hs(qtilT[2 * rp + rr], h),
                                             start=True, stop=True)
                        e = work.tile([128, 2 * I], dt.bfloat16, tag="sbev",
                                      name="sbev")
                        nc.scalar.copy(e, ps)
                        sbw.append(e)
                    u01 = work.tile([128, I], dt.bfloat16, tag="u01", name="u01",
                                    bufs=2)
                    u23 = work.tile([128, I], dt.bfloat16, tag="u23", name="u23",
                                    bufs=2)
                    t0 = work.tile([128, I], dt.bfloat16, tag="t0", name="t0",
                                   bufs=2)
                    t1 = work.tile([128, I], dt.bfloat16, tag="t1", name="t1",
                                   bufs=2)
                    nc.vector.tensor_mul(u01, sbw[0][:, 0:I], mT[0][jc])
                    nc.vector.tensor_mul(t0, sbw[0][:, I:2 * I], mT[1][jc])
                    nc.vector.tensor_mul(u23, sbw[1][:, 0:I], mT[2][jc])
                    nc.vector.tensor_mul(t1, sbw[1][:, I:2 * I], mT[3][jc])
                    nc.vector.tensor_add(u01, u01, t0)
                    nc.vector.tensor_add(u23, u23, t1)
                    res = ph.tile([128, I], dt.bfloat16, tag="res", name="res",
                                  bufs=9)
                    nc.vector.tensor_add(res, u01, u23)
                    res_h.append(res)
                    for ib in range(NIB):
                        nc.tensor.matmul(rsP[:, ib:ib + 1],
                                         res[:, ib * 128:(ib + 1) * 128], onesCol,
                                         start=(h == 0 and jc == 0),
                                         stop=(h == H - 1 and jc == NJC - 1))
                P_h = []
                for jc in range(NJC):
                    p = ph.tile([128, I], dt.bfloat16, tag="P", name="P")
                    nc.scalar.activation(p, res_h[jc], AF.Exp)
                    P_h.append(p)
                    for ib in range(NIB):
                        nc.tensor.matmul(zP[:, ib:ib + 1],
                                         p[:, ib * 128:(ib + 1) * 128], onesCol,
                                         start=(jc == 0), stop=(jc == NJC - 1))
                # 1/Z for this head
                zh = work.tile([128, NIB], dt.float32, tag="zh", name="zh")
                nc.vector.tensor_copy(zh, zP)
                zrec = work.tile([128, NIB], dt.float32, tag="zrec", name="zrec",
                                 bufs=2)
                nc.vector.reciprocal(zrec, zh)
                # ---- phase 2: per (s,r) masked scores + aggregation ----
                for jc in range(NJC):
                    W = []
                    for r in range(R):
                        w = work.tile([128, I], dt.bfloat16, tag=f"W{r}",
                                      name=f"W{r}", bufs=2)
                        nc.vector.tensor_mul(w, P_h[jc], mT[r][jc])
                        W.append(w)
                    for s in range(T):
                        for rp in range(2):
                            ps = psM.tile([128, 2 * I], dt.float32, tag="sc",
                                          name="sc")
                            for rr in range(2):
                                nc.tensor.matmul(ps[:, rr * I:(rr + 1) * I],
                                                 hs(ktilT[s], h, jc * 128, (jc + 1) * 128),
                                                 hs(qtilT[2 * rp + rr], h),
                                                 start=True, stop=True)
                            direct = (s == 2 and rp == 1)
                            if direct:
                                src_t = ps
                            else:
                                src_t = work.tile([128, 2 * I], dt.bfloat16,
                                                  tag="sev", name="sev")
                                nc.scalar.copy(src_t, ps)
                            for rr in range(2):
                                r = 2 * rp + rr
                                x = work.tile([128, I], dt.bfloat16, tag="X",
                                              name="X")
                                nc.vector.tensor_mul(x, src_t[:, rr * I:(rr + 1) * I],
                                                     W[r])
                                for ib in range(NIB):
                                    nc.tensor.matmul(
                                        aggp[:, (r * NIB + ib) * 32:(r * NIB + ib) * 32 + 32],
                                        x[:, ib * 128:(ib + 1) * 128],
                                        vt[s][jc][:, h * 32:h * 32 + 32],
                                        start=(jc == 0 and s == 0),
                                        stop=(jc == NJC - 1 and s == T - 1))
                # normalize by 1/Z while evacuating this head's aggregation
                for r in range(R):
                    for ib in range(NIB):
                        nc.vector.tensor_scalar(
                            out=aggn[r][ib][:, h * 32:h * 32 + 32],
                            in0=aggp[:, (r * NIB + ib) * 32:(r * NIB + ib) * 32 + 32],
                            scalar1=zrec[:, ib:ib + 1], scalar2=None, op0=ALU.mult)

            # row-sum gate: zero rows whose logit-sum <= 1e-6
            rs_sb = work.tile([128, NIB], dt.float32, tag="rssb", name="rssb")
            nc.vector.tensor_copy(rs_sb, rsP)
            cond = const.tile([128, NIB], dt.float32, tag="cond", name="cond")
            nc.vector.tensor_scalar(out=cond, in0=rs_sb, scalar1=1e-6,
                                    scalar2=None, op0=ALU.is_gt)

        # ---------------- stage D: rel_msg, gelu, linear, LN, type mix ----------------
        with tc.tile_pool(name="psT", bufs=2, space="PSUM") as psT, \
             tc.tile_pool(name="psB", bufs=2, space="PSUM") as psB, \
             tc.tile_pool(name="psO", bufs=2, space="PSUM") as psO:
            # apply gate, transpose agg'_r -> [(h,d'), i]
            aggTin = [[None] * OC for _ in range(R)]
            for r in range(R):
                for kc in range(OC):
                    aggTin[r][kc] = const.tile([128, I], dt.bfloat16,
                                               tag=f"aggT{r}_{kc}",
                                               name=f"aggT{r}_{kc}")
            for r in range(R):
                for ib in range(NIB):
                    ac = work.tile([128, OUT], dt.bfloat16, tag="aggc", name="aggc",
                                   bufs=2)
                    nc.vector.tensor_scalar(out=ac, in0=aggn[r][ib],
                                            scalar1=cond[:, ib:ib + 1],
                                            scalar2=None, op0=ALU.mult)
                    for kc in range(OC):
                        pt = psT.tile([128, 128], dt.bfloat16, tag="tp", name="tp")
                        nc.tensor.transpose(pt, ac[:, kc * 128:(kc + 1) * 128],
                                            ident)
                        nc.scalar.copy(aggTin[r][kc][:, ib * 128:(ib + 1) * 128], pt)
            # aggT = sum_r Bbd_r-rotated aggregate, then gelu -> gT
            gT = []
            for ot in range(OC):
                pb = psB.tile([128, I], dt.float32, tag="aggTp", name="aggTp")
                for r in range(R):
                    for kc in range(OC):
                        nc.tensor.matmul(pb,
                                         bbd_t[r][kc][:, ot * 128:(ot + 1) * 128],
                                         aggTin[r][kc],
                                         start=(r == 0 and kc == 0),
                                         stop=(r == R - 1 and kc == OC - 1))
                g = const.tile([128, I], dt.bfloat16, tag=f"gT{ot}", name=f"gT{ot}")
                nc.scalar.activation(g, pb, AF.Gelu)
                gT.append(g)
            # per-type head: trans = g @ Wa_t + ba_t ; blend ; LN ; type mix
            for ib in range(NIB):
                o_prev = None
                for t_ in range(T):
                    po = psO.tile([128, OUT], dt.float32, tag="trp", name="trp")
                    for kc in range(OC):
                        nc.tensor.matmul(po, gT[kc][:, ib * 128:(ib + 1) * 128],
                                         wa_t[t_][kc], start=(kc == 0), stop=False)
                    nc.tensor.matmul(po, onesRow[0:1, ib * 128:(ib + 1) * 128],
                                     wa_t[t_][2], start=False, stop=True)
                    rt = work.tile([128, OUT], dt.bfloat16, tag="rt", name="rt",
                                   bufs=2)
                    nc.vector.scalar_tensor_tensor(out=rt, in0=po,
                                                   scalar=alphB[t_], in1=xa[t_][ib],
                                                   op0=ALU.mult, op1=ALU.add)
                    st6 = work.tile([128, 6], dt.float32, tag="st6", name="st6",
                                    bufs=2)
                    nc.vector.bn_stats(out=st6, in_=rt)
                    mv = work.tile([128, 2], dt.float32, tag="mv", name="mv",
                                   bufs=2)
                    nc.vector.bn_aggr(out=mv, in_=st6)
                    ve = work.tile([128, 1], dt.float32, tag="ve", name="ve",
                                   bufs=2)
                    nc.vector.tensor_scalar(out=ve, in0=mv[:, 1:2], scalar1=LN_EPS,
                                            scalar2=None, op0=ALU.add)
                    sd = work.tile([128, 1], dt.float32, tag="sd", name="sd",
                                   bufs=2)
                    nc.scalar.sqrt(sd, ve)
                    rstd = work.tile([128, 1], dt.float32, tag="rstd", name="rstd",
                                     bufs=2)
                    nc.vector.reciprocal(rstd, sd)
                    cen = work.tile([128, OUT], dt.bfloat16, tag="cen", name="cen",
                                    bufs=2)
                    nc.vector.tensor_scalar(out=cen, in0=rt, scalar1=mv[:, 0:1],
                                            scalar2=None, op0=ALU.subtract)
                    v1 = work.tile([128, OUT], dt.bfloat16, tag="v1", name="v1",
                                   bufs=2)
                    nc.vector.scalar_tensor_tensor(out=v1, in0=cen, scalar=rstd,
                                                   in1=gB[t_], op0=ALU.mult,
                                                   op1=ALU.mult)
                    tycol = tyI_t[ib][:, t_:t_ + 1]
                    if t_ == 0:
                        ob = work.tile([128, OUT], dt.bfloat16, tag="ob0",
                                       name="ob0", bufs=2)
                        nc.vector.tensor_scalar(out=ob, in0=bB[t_], scalar1=tycol,
                                                scalar2=None, op0=ALU.mult)
                    else:
                        ob = work.tile([128, OUT], dt.bfloat16, tag=f"ob{t_}",
                                       name=f"ob{t_}", bufs=2)
                        nc.vector.scalar_tensor_tensor(out=ob, in0=bB[t_],
                                                       scalar=tycol, in1=o_prev,
                                                       op0=ALU.mult, op1=ALU.add)
                    odt = dt.float32 if t_ == T - 1 else dt.bfloat16
                    onew = work.tile([128, OUT], odt, tag=f"oacc{t_}",
                                     name=f"oacc{t_}", bufs=2)
                    nc.vector.scalar_tensor_tensor(out=onew, in0=v1, scalar=tycol,
                                                   in1=ob, op0=ALU.mult, op1=ALU.add)
                    o_prev = onew
                nc.sync.dma_start(out=outP[ib * 128:(ib + 1) * 128, :], in_=o_prev)

    nc.compile()
    _built["nc"] = nc
    return nc


def _host_prep(inputs):
    """Build the 8 per-core input dicts (numpy, host-side sharding/casts)."""
    f32 = np.float32
    x_all = np.asarray(inputs["node_features"], f32)
    ty_all = np.asarray(inputs["node_types_soft"], f32)
    adj_all = np.asarray(inputs["adj_matrix_soft"], f32)
    erel_all = np.asarray(inputs["edge_types_soft"], f32)
    Wq = np.asarray(inputs["Wq"], f32); bq = np.asarray(inputs["bq"], f32)
    Wk = np.asarray(inputs["Wk"], f32); bk = np.asarray(inputs["bk"], f32)
    Wv = np.asarray(inputs["Wv"], f32); bv = np.asarray(inputs["bv"], f32)
    Wa = np.asarray(inputs["Wa"], f32); ba = np.asarray(inputs["ba"], f32)
    rel_pri = np.asarray(inputs["rel_pri"], f32)
    rel_att = np.asarray(inputs["rel_att"], f32)
    rel_msg = np.asarray(inputs["rel_msg"], f32)
    skip = np.asarray(inputs["skip"], f32)
    lng = np.asarray(inputs["ln_gamma"], f32)
    lnb = np.asarray(inputs["ln_beta"], f32)

    sqrt_dk = math.sqrt(DK)
    abd = np.zeros((R, OUT, OUT), f32)
    bbd = np.zeros((R, OUT, OUT), f32)
    for r in range(R):
        for h in range(H):
            sl = slice(h * DK, (h + 1) * DK)
            abd[r, sl, sl] = rel_att[r, h].T * (rel_pri[r, h] / sqrt_dk)
            bbd[r, sl, sl] = rel_msg[r, h]
    alpha = 1.0 / (1.0 + np.exp(-skip))
    alph = np.stack([alpha, 1.0 - alpha]).astype(f32)

    def bf(a):
        return np.ascontiguousarray(a.astype(BF16))

    wq_aug = bf(np.concatenate([Wq, bq[:, None, :]], axis=1))
    wk_aug = bf(np.concatenate([Wk, bk[:, None, :]], axis=1))
    wv_aug = bf(np.concatenate([Wv, bv[:, None, :]], axis=1))
    wa_aug = bf(np.concatenate([Wa, ba[:, None, :]], axis=1))
    abd_b, bbd_b = bf(abd), bf(bbd)
    lng_b, lnb_b = bf(lng), bf(lnb)

    in_maps = []
    for c in range(NCORES):
        b, half = c // 2, c % 2
        isl = slice(half * I, half * I + I)
        x = x_all[b]
        ty = ty_all[b]
        xT_aug = np.concatenate([x.T, np.ones((1, N), f32)], axis=0)
        xTs_aug = np.stack([
            np.concatenate([(x * ty[:, s:s + 1]).T, ty[None, :, s]], axis=0)
            for s in range(T)])
        in_maps.append({
            "xT": bf(xT_aug),
            "xTI": bf(xT_aug[:, isl]),
            "xI": bf(x[isl]),
            "xTs": bf(xTs_aug),
            "xTsI": bf(xTs_aug[:, :, isl]),
            "typesI": np.ascontiguousarray(ty[isl]),
            "adjT": bf(adj_all[b][isl, :].T),
            "erelT": bf(erel_all[b][isl, :, :].transpose(2, 1, 0)),
            "wq": wq_aug, "wk": wk_aug, "wv": wv_aug, "wa": wa_aug,
            "abd": abd_b, "bbd": bbd_b, "lng": lng_b, "lnb": lnb_b,
            "alph": alph,
        })
    return in_maps


def kernel(**inputs):
    from concourse.bass_utils import run_bass_kernel_spmd
    nc = _build_nc()
    in_maps = _host_prep(inputs)
    res = run_bass_kernel_spmd(nc, in_maps, core_ids=list(range(NCORES)))
    out = np.zeros((B, N, OUT), np.float32)
    for c in range(NCORES):
        b, half = c // 2, c % 2
        out[b, half * I:half * I + I, :] = res.results[c]["out"]
    return out


# revision 10
# speedup vs baseline: 2.3556x; 2.3556x over previous
"""Trainium2 Bass kernel for nn_DifferentiableDenseHGTConv.

Self-contained: takes FULL inputs as numpy arrays, shards batch x target-row
halves across 8 NeuronCores (core c -> batch c//2, row-half c%2), runs one
SPMD Bass/Tile kernel, gathers the full [4,1024,256] fp32 output.

Per-core dataflow (I = 512 target rows, N = 1024 sources, H=8 heads, DK=32):
  All [N,N,H]-scale tensors live in [j(source)-partition, i(target)-free]
  orientation so the attention-weighted aggregation matmuls contract over j
  in PE partitions.  Logits res_att^T = sum_r m_r (.) (K~bar x Q~_r) are
  built by PE matmuls (bf16) + DVE masked sums; softmax denominators and the
  row-sum gate are partition reductions done as PE ones-matmuls; the
  normalization (cond/Z) is folded to the aggregation epilogue.  The
  relation/type structure is factorized so rel_att (+pri/sqrt(dk)) is folded
  into per-relation queries Q~_r and rel_msg is applied once at the end on
  the [I,256] aggregate (block-diagonal 256x256 matmuls).
"""
import math
import numpy as np
import ml_dtypes

T, R, H, DK = 3, 4, 8, 32
OUT = H * DK          # 256
IN = 256
B, N = 4, 1024
I = 512               # target rows per core
NCORES = 8
LN_EPS = 1e-5
BF16 = ml_dtypes.bfloat16

_built = {}


def _build_nc():
    """Build + compile the SPMD Bass module once per process."""
    if "nc" in _built:
        return _built["nc"]

    from contextlib import ExitStack
    import concourse.bass as bass
    import concourse.tile as tile
    from concourse import bacc, mybir
    from concourse.masks import make_identity

    dt = mybir.dt
    AF = mybir.ActivationFunctionType
    ALU = mybir.AluOpType

    nc = bacc.Bacc("TRN2", target_bir_lowering=False, debug=False,
                   enable_asserts=True, num_devices=NCORES)

    # ---------------- DRAM parameters (per core) ----------------
    xT = nc.declare_dram_parameter("xT", [IN + 1, N], dt.bfloat16, isOutput=False)
    xTI = nc.declare_dram_parameter("xTI", [IN + 1, I], dt.bfloat16, isOutput=False)
    xI = nc.declare_dram_parameter("xI", [I, IN], dt.bfloat16, isOutput=False)
    xTs = nc.declare_dram_parameter("xTs", [T, IN + 1, N], dt.bfloat16, isOutput=False)
    xTsI = nc.declare_dram_parameter("xTsI", [T, IN + 1, I], dt.bfloat16, isOutput=False)
    typesI = nc.declare_dram_parameter("typesI", [I, T], dt.float32, isOutput=False)
    adjT = nc.declare_dram_parameter("adjT", [N, I], dt.bfloat16, isOutput=False)
    erelT = nc.declare_dram_parameter("erelT", [R, N, I], dt.bfloat16, isOutput=False)
    wq = nc.declare_dram_parameter("wq", [T, IN + 1, OUT], dt.bfloat16, isOutput=False)
    wk = nc.declare_dram_parameter("wk", [T, IN + 1, OUT], dt.bfloat16, isOutput=False)
    wv = nc.declare_dram_parameter("wv", [T, IN + 1, OUT], dt.bfloat16, isOutput=False)
    abd = nc.declare_dram_parameter("abd", [R, OUT, OUT], dt.bfloat16, isOutput=False)
    bbd = nc.declare_dram_parameter("bbd", [R, OUT, OUT], dt.bfloat16, isOutput=False)
    wa = nc.declare_dram_parameter("wa", [T, OUT + 1, OUT], dt.bfloat16, isOutput=False)
    lng = nc.declare_dram_parameter("lng", [T, OUT], dt.bfloat16, isOutput=False)
    lnb = nc.declare_dram_parameter("lnb", [T, OUT], dt.bfloat16, isOutput=False)
    alph = nc.declare_dram_parameter("alph", [2, T], dt.float32, isOutput=False)
    outP = nc.declare_dram_parameter("out", [I, OUT], dt.float32, isOutput=True)

    def bcast(row_ap, parts=128):
        # DMA access pattern replicating one DRAM row across `parts` partitions
        return bass.AP(tensor=row_ap.tensor, offset=row_ap.offset,
                       ap=[[0, parts]] + list(row_ap.ap[1:]))

    NJC = N // 128    # 8 source chunks
    NIB = I // 128    # 4 target blocks
    OC = OUT // 128   # 2 output-dim chunks

    with ExitStack() as ctx:
        tc = ctx.enter_context(tile.TileContext(nc))
        const = ctx.enter_context(tc.tile_pool(name="const", bufs=1))
        work = ctx.enter_context(tc.tile_pool(name="work", bufs=3))

        # ---------------- persistent constants ----------------
        ident = const.tile([128, 128], dt.bfloat16, tag="ident", name="ident")
        make_identity(nc, ident)
        onesCol = const.tile([128, 1], dt.bfloat16, tag="onescol", name="onescol")
        nc.vector.memset(onesCol, 1.0)
        onesRow = const.tile([1, I], dt.bfloat16, tag="onesrow", name="onesrow")
        nc.vector.memset(onesRow, 1.0)

        xI_t = []
        for ib in range(NIB):
            tl = const.tile([128, IN], dt.bfloat16, tag=f"xI{ib}", name=f"xI{ib}")
            nc.sync.dma_start(out=tl, in_=xI[ib * 128:(ib + 1) * 128, :])
            xI_t.append(tl)
        tyI_t = []
        for ib in range(NIB):
            tl = const.tile([128, T], dt.float32, tag=f"tyI{ib}", name=f"tyI{ib}")
            nc.sync.dma_start(out=tl, in_=typesI[ib * 128:(ib + 1) * 128, :])
            tyI_t.append(tl)
        gB, bB = [], []
        for t_ in range(T):
            tl = const.tile([128, OUT], dt.bfloat16, tag=f"gB{t_}", name=f"gB{t_}")
            nc.gpsimd.dma_start(out=tl, in_=bcast(lng[t_:t_ + 1, :]))
            gB.append(tl)
            tl = const.tile([128, OUT], dt.bfloat16, tag=f"bB{t_}", name=f"bB{t_}")
            nc.gpsimd.dma_start(out=tl, in_=bcast(lnb[t_:t_ + 1, :]))
            bB.append(tl)
        alphB, alph1mB = [], []
        for t_ in range(T):
            tl = const.tile([128, 1], dt.float32, tag=f"alB{t_}", name=f"alB{t_}")
            nc.gpsimd.dma_start(out=tl, in_=bcast(alph[0:1, t_:t_ + 1]))
            alphB.append(tl)
            tl = const.tile([128, 1], dt.float32, tag=f"al1B{t_}", name=f"al1B{t_}")
            nc.gpsimd.dma_start(out=tl, in_=bcast(alph[1:2, t_:t_ + 1]))
            alph1mB.append(tl)

        def load_w3(pool, param, name, klen=IN):
            # [klen+1, OUT] augmented weight -> [128,256] x(klen//128) + [1,256]
            out = []
            for t_ in range(T):
                tls = []
                for kc in range(klen // 128):
                    tl = pool.tile([128, OUT], dt.bfloat16, tag=f"{name}{t_}_{kc}",
                                   name=f"{name}{t_}_{kc}")
                    nc.sync.dma_start(out=tl, in_=param[t_, kc * 128:(kc + 1) * 128, :])
                    tls.append(tl)
                tl = pool.tile([1, OUT], dt.bfloat16, tag=f"{name}{t_}_a",
                               name=f"{name}{t_}_a")
                nc.sync.dma_start(out=tl, in_=param[t_, klen:klen + 1, :])
                tls.append(tl)
                out.append(tls)
            return out

        wa_t = load_w3(const, wa, "wa", klen=OUT)
        bbd_t = []
        for r in range(R):
            tls = []
            for kc in range(OC):
                tl = const.tile([128, OUT], dt.bfloat16, tag=f"bbd{r}_{kc}",
                                name=f"bbd{r}_{kc}")
                nc.sync.dma_start(out=tl, in_=bbd[r, kc * 128:(kc + 1) * 128, :])
                tls.append(tl)
            bbd_t.append(tls)

        # Head-sliced PE operands are packed 3 heads per tile (32-row bands at
        # base partitions 0/32/64 -- the only legal matmul base partitions).
        def hs(tiles, h, lo=None, hi=None):
            b = (h % 3) * 32
            if lo is None:
                return tiles[h // 3][b:b + 32, :]
            return tiles[h // 3][b:b + 32, lo:hi]

        def hpack_alloc(name_, width):
            return [const.tile([96, width], dt.bfloat16, tag=f"{name_}0", name=f"{name_}0"),
                    const.tile([96, width], dt.bfloat16, tag=f"{name_}1", name=f"{name_}1"),
                    const.tile([64, width], dt.bfloat16, tag=f"{name_}2", name=f"{name_}2")]

        # ---------------- stage A: projections ----------------
        # Host pre-scales x by each soft-type column (aug row = types row), so
        # K~_s^T / QbarT / KbarT are plain augmented matmuls with PSUM
        # accumulation doing the type mixing.
        ktilT = []                                # per s: 3 packed tiles [., N]
        kbarT = hpack_alloc("kbar", N)
        qtilT = []                                # per r: 3 packed tiles [., I]
        vt = [[None] * NJC for _ in range(T)]     # Vt_s [N, 256]
        with tc.tile_pool(name="sA", bufs=1) as sA:
            xT_t = []
            for kc in range(2):
                tl = sA.tile([128, N], dt.bfloat16, tag=f"xT{kc}", name=f"xT{kc}")
                nc.sync.dma_start(out=tl, in_=xT[kc * 128:(kc + 1) * 128, :])
                xT_t.append(tl)
            xT_t.append(sA.tile([1, N], dt.bfloat16, tag="xT2", name="xT2"))
            nc.sync.dma_start(out=xT_t[2], in_=xT[IN:IN + 1, :])
            wq_t = load_w3(sA, wq, "wq")
            wk_t = load_w3(sA, wk, "wk")
            wv_t = load_w3(sA, wv, "wv")
            abd_t = []
            for r in range(R):
                tls = []
                for kc in range(OC):
                    tl = sA.tile([128, OUT], dt.bfloat16, tag=f"abd{r}_{kc}",
                                 name=f"abd{r}_{kc}")
                    nc.sync.dma_start(out=tl, in_=abd[r, kc * 128:(kc + 1) * 128, :])
                    tls.append(tl)
                abd_t.append(tls)

            # K side: K~T_s and KbarT via PSUM accumulation over (s, kc)
            for s in range(T):
                ktilT.append(hpack_alloc(f"ktil{s}_", N))
            with tc.tile_pool(name="psK", bufs=2, space="PSUM") as psK, \
                 tc.tile_pool(name="psKb", bufs=1, space="PSUM") as psKb:
                kbarP = [psKb.tile([128, N], dt.float32, tag=f"kbarP{oc}",
                                   name=f"kbarP{oc}") for oc in range(OC)]
                for s in range(T):
                    xs_t = []
                    for kc in range(2):
                        tl = sA.tile([128, N], dt.bfloat16, tag=f"xs{kc}",
                                     name=f"xs{kc}", bufs=1)
                        nc.sync.dma_start(out=tl, in_=xTs[s, kc * 128:(kc + 1) * 128, :])
                        xs_t.append(tl)
                    tl = sA.tile([1, N], dt.bfloat16, tag="xs2", name="xs2", bufs=1)
                    nc.sync.dma_start(out=tl, in_=xTs[s, IN:IN + 1, :])
                    xs_t.append(tl)
                    for oc in range(OC):
                        pa = psK.tile([128, N], dt.float32, tag="pA", name="pA")
                        for kc in range(3):
                            for nh in range(2):
                                nsl = slice(nh * 512, (nh + 1) * 512)
                                nc.tensor.matmul(pa[:, nsl],
                                                 wk_t[s][kc][:, oc * 128:(oc + 1) * 128],
                                                 xs_t[kc][:, nsl],
                                                 start=(kc == 0), stop=(kc == 2))
                                nc.tensor.matmul(kbarP[oc][:, nsl],
                                                 wk_t[s][kc][:, oc * 128:(oc + 1) * 128],
                                                 xs_t[kc][:, nsl],
                                                 start=(s == 0 and kc == 0),
                                                 stop=(s == T - 1 and kc == 2))
                        kt = sA.tile([128, N], dt.bfloat16, tag="ktev", name="ktev",
                                     bufs=1)
                        nc.scalar.copy(kt, pa)
                        for hh in range(4):
                            h = oc * 4 + hh
                            nc.sync.dma_start(out=hs(ktilT[s], h),
                                              in_=kt[hh * 32:(hh + 1) * 32, :])
                for oc in range(OC):
                    kb = sA.tile([128, N], dt.bfloat16, tag="kbev", name="kbev",
                                 bufs=1)
                    nc.scalar.copy(kb, kbarP[oc])
                    for hh in range(4):
                        h = oc * 4 + hh
                        nc.sync.dma_start(out=hs(kbarT, h),
                                          in_=kb[hh * 32:(hh + 1) * 32, :])

            # Q side: QbarT via PSUM accumulation over (t, kc), then rel_att fold
            qbarT = []
            with tc.tile_pool(name="psQ", bufs=2, space="PSUM") as psQ:
                for oc in range(OC):
                    pa = psQ.tile([128, I], dt.float32, tag="pQb", name="pQb")
                    for t_ in range(T):
                        xsi_t = []
                        for kc in range(2):
                            tl = sA.tile([128, I], dt.bfloat16, tag=f"xsi{kc}",
                                         name=f"xsi{kc}", bufs=1)
                            nc.sync.dma_start(out=tl,
                                              in_=xTsI[t_, kc * 128:(kc + 1) * 128, :])
                            xsi_t.append(tl)
                        tl = sA.tile([1, I], dt.bfloat16, tag="xsi2", name="xsi2",
                                     bufs=1)
                        nc.sync.dma_start(out=tl, in_=xTsI[t_, IN:IN + 1, :])
                        xsi_t.append(tl)
                        for kc in range(3):
                            nc.tensor.matmul(pa,
                                             wq_t[t_][kc][:, oc * 128:(oc + 1) * 128],
                                             xsi_t[kc], start=(t_ == 0 and kc == 0),
                                             stop=(t_ == T - 1 and kc == 2))
                    qb = sA.tile([128, I], dt.bfloat16, tag=f"qbar{oc}",
                                 name=f"qbar{oc}")
                    nc.scalar.copy(qb, pa)
                    qbarT.append(qb)
                for r in range(R):
                    qtilT.append(hpack_alloc(f"qtil{r}_", I))
                    for ot in range(OC):
                        pa = psQ.tile([128, I], dt.float32, tag="pQb", name="pQb")
                        for kc in range(OC):
                            nc.tensor.matmul(pa,
                                             abd_t[r][kc][:, ot * 128:(ot + 1) * 128],
                                             qbarT[kc], start=(kc == 0), stop=(kc == 1))
                        qt = sA.tile([128, I], dt.bfloat16, tag="qt128", name="qt128",
                                     bufs=1)
                        nc.scalar.copy(qt, pa)
                        for hh in range(4):
                            h = ot * 4 + hh
                            nc.sync.dma_start(out=hs(qtilT[r], h),
                                              in_=qt[hh * 32:(hh + 1) * 32, :])

            # V side: Vt_s[j, o]
            with tc.tile_pool(name="psV", bufs=3, space="PSUM") as psV:
                for s in range(T):
                    for jb in range(NJC):
                        pa = psV.tile([128, OUT], dt.float32, tag="pV", name="pV")
                        for kc in range(3):
                            nc.tensor.matmul(pa,
                                             xT_t[kc][:, jb * 128:(jb + 1) * 128],
                                             wv_t[s][kc], start=(kc == 0), stop=(kc == 2))
                        v = const.tile([128, OUT], dt.bfloat16, tag=f"vt{s}_{jb}",
                                       name=f"vt{s}_{jb}")
                        nc.scalar.copy(v, pa)
                        vt[s][jb] = v

        # masks m_r^T[j, i] = adjT (.) erelT_r
        mT = [[None] * NJC for _ in range(R)]
        dma_q = [nc.sync, nc.scalar]
        for jc in range(NJC):
            at = work.tile([128, I], dt.bfloat16, tag="adj_in", name="adj_in", bufs=2)
            dma_q[jc % 2].dma_start(out=at, in_=adjT[jc * 128:(jc + 1) * 128, :])
            for r in range(R):
                et = work.tile([128, I], dt.bfloat16, tag="erel_in", name="erel_in",
                               bufs=4)
                dma_q[(jc + r) % 2].dma_start(out=et,
                                              in_=erelT[r, jc * 128:(jc + 1) * 128, :])
                m = const.tile([128, I], dt.bfloat16, tag=f"m{r}_{jc}",
                               name=f"m{r}_{jc}")
                nc.vector.tensor_mul(m, at, et)
                mT[r][jc] = m

        # (1-alpha_t) * x residual tiles
        xa = [[None] * NIB for _ in range(T)]
        for t_ in range(T):
            for ib in range(NIB):
                tl = const.tile([128, IN], dt.bfloat16, tag=f"xa{t_}_{ib}",
                                name=f"xa{t_}_{ib}")
                nc.vector.tensor_scalar(out=tl, in0=xI_t[ib],
                                        scalar1=alph1mB[t_], scalar2=None,
                                        op0=ALU.mult)
                xa[t_][ib] = tl

        # ---------------- main h-loop ----------------
        aggn = [[None] * NIB for _ in range(R)]  # agg'_r / Z, [I, (h,d')] bf16
        for r in range(R):
            for ib in range(NIB):
                aggn[r][ib] = const.tile([128, OUT], dt.bfloat16,
                                         tag=f"aggn{r}_{ib}", name=f"aggn{r}_{ib}")

        with tc.tile_pool(name="psM", bufs=2, space="PSUM") as psM, \
             tc.tile_pool(name="psG", bufs=1, space="PSUM") as psG, \
             tc.tile_pool(name="psZ", bufs=2, space="PSUM") as psZ, \
             tc.tile_pool(name="psR", bufs=1, space="PSUM") as psR, \
             tc.tile_pool(name="ph", bufs=10) as ph:

            rsP = psR.tile([128, NIB], dt.float32, tag="rs", name="rs")
            for h in range(H):
                zP = psZ.tile([128, NIB], dt.float32, tag="z", name="z")
                aggp = psG.tile([128, 512], dt.float32, tag="aggp", name="aggp")
                # ---- phase 1: logits (wide paired evacs), batched exp ----
                res_h = []
                for jc in range(NJC):
                    sbw = []
                    for rp in range(2):
                        ps = psM.tile([128, 2 * I], dt.float32, tag="sc", name="sc")
                        for rr in range(2):
                            nc.tensor.matmul(ps[:, rr * I:(rr + 1) * I],
                                             hs(kbarT, h, jc * 128, (jc + 1) * 128),
                                             hs(qtilT[2 * rp + rr], h),
                                             start=True, stop=True)
                        e = work.tile([128, 2 * I], dt.bfloat16, tag="sbev",
                                      name="sbev")
                        nc.scalar.copy(e, ps)
                        sbw.append(e)
                    u01 = work.tile([128, I], dt.bfloat16, tag="u01", name="u01",
                                    bufs=2)
                    u23 = work.tile([128, I], dt.bfloat16, tag="u23", name="u23",
                                    bufs=2)
                    t0 = work.tile([128, I], dt.bfloat16, tag="t0", name="t0",
                                   bufs=2)
                    t1 = work.tile([128, I], dt.bfloat16, tag="t1", name="t1",
                                   bufs=2)
                    nc.vector.tensor_mul(u01, sbw[0][:, 0:I], mT[0][jc])
                    nc.vector.tensor_mul(t0, sbw[0][:, I:2 * I], mT[1][jc])
                    nc.vector.tensor_mul(u23, sbw[1][:, 0:I], mT[2][jc])
                    nc.vector.tensor_mul(t1, sbw[1][:, I:2 * I], mT[3][jc])
                    nc.vector.tensor_add(u01, u01, t0)
                    nc.vector.tensor_add(u23, u23, t1)
                    res = ph.tile([128, I], dt.bfloat16, tag="res", name="res",
                                  bufs=9)
                    nc.vector.tensor_add(res, u01, u23)
                    res_h.append(res)
                    for ib in range(NIB):
                        nc.tensor.matmul(rsP[:, ib:ib + 1],
                                         res[:, ib * 128:(ib + 1) * 128], onesCol,
                                         start=(h == 0 and jc == 0),
                                         stop=(h == H - 1 and jc == NJC - 1))
                P_h = []
                for jc in range(NJC):
                    p = ph.tile([128, I], dt.bfloat16, tag="P", name="P")
                    nc.scalar.activation(p, res_h[jc], AF.Exp)
                    P_h.append(p)
                    for ib in range(NIB):
                        nc.tensor.matmul(zP[:, ib:ib + 1],
                                         p[:, ib * 128:(ib + 1) * 128], onesCol,
                                         start=(jc == 0), stop=(jc == NJC - 1))
                # 1/Z for this head
                zh = work.tile([128, NIB], dt.float32, tag="zh", name="zh")
                nc.vector.tensor_copy(zh, zP)
                zrec = work.tile([128, NIB], dt.float32, tag="zrec", name="zrec",
                                 bufs=2)
                nc.vector.reciprocal(zrec, zh)
                # ---- phase 2: per (s,r) masked scores + aggregation ----
                for jc in range(NJC):
                    W = []
                    for r in range(R):
                        w = work.tile([128, I], dt.bfloat16, tag=f"W{r}",
                                      name=f"W{r}", bufs=2)
                        nc.vector.tensor_mul(w, P_h[jc], mT[r][jc])
                        W.append(w)
                    for s in range(T):
                        for rp in range(2):
                            ps = psM.tile([128, 2 * I], dt.float32, tag="sc",
                                          name="sc")
                            for rr in range(2):
                                nc.tensor.matmul(ps[:, rr * I:(rr + 1) * I],
                                                 hs(ktilT[s], h, jc * 128, (jc + 1) * 128),
                                                 hs(qtilT[2 * rp + rr], h),
                                                 start=True, stop=True)
                            direct = (s == 2 and rp == 1)
                            if direct:
                                src_t = ps
                            else:
                                src_t = work.tile([128, 2 * I], dt.bfloat16,
                                                  tag="sev", name="sev")
                                nc.scalar.copy(src_t, ps)
                            for rr in range(2):
                                r = 2 * rp + rr
                                x = work.tile([128, I], dt.bfloat16, tag="X",
                                              name="X")
                                nc.vector.tensor_mul(x, src_t[:, rr * I:(rr + 1) * I],
                                                     W[r])
                                for ib in range(NIB):
                                    nc.tensor.matmul(
                                        aggp[:, (r * NIB + ib) * 32:(r * NIB + ib) * 32 + 32],
                                        x[:, ib * 128:(ib + 1) * 128],
                                        vt[s][jc][:, h * 32:h * 32 + 32],
                                        start=(jc == 0 and s == 0),
                                        stop=(jc == NJC - 1 and s == T - 1))
                # normalize by 1/Z while evacuating this head's aggregation
                for r in range(R):
                    for ib in range(NIB):
                        nc.vector.tensor_scalar(
                            out=aggn[r][ib][:, h * 32:h * 32 + 32],
                            in0=aggp[:, (r * NIB + ib) * 32:(r * NIB + ib) * 32 + 32],
                            scalar1=zrec[:, ib:ib + 1], scalar2=None, op0=ALU.mult)

            # row-sum gate: zero rows whose logit-sum <= 1e-6
            rs_sb = work.tile([128, NIB], dt.float32, tag="rssb", name="rssb")
            nc.vector.tensor_copy(rs_sb, rsP)
            cond = const.tile([128, NIB], dt.float32, tag="cond", name="cond")
            nc.vector.tensor_scalar(out=cond, in0=rs_sb, scalar1=1e-6,
                                    scalar2=None, op0=ALU.is_gt)

        # ---------------- stage D: rel_msg, gelu, linear, LN, type mix ----------------
        with tc.tile_pool(name="psT", bufs=2, space="PSUM") as psT, \
             tc.tile_pool(name="psB", bufs=2, space="PSUM") as psB, \
             tc.tile_pool(name="psO", bufs=2, space="PSUM") as psO:
            # apply gate, transpose agg'_r -> [(h,d'), i]
            aggTin = [[None] * OC for _ in range(R)]
            for r in range(R):
                for kc in range(OC):
                    aggTin[r][kc] = const.tile([128, I], dt.bfloat16,
                                               tag=f"aggT{r}_{kc}",
                                               name=f"aggT{r}_{kc}")
            for r in range(R):
                for ib in range(NIB):
                    ac = work.tile([128, OUT], dt.bfloat16, tag="aggc", name="aggc",
                                   bufs=2)
                    nc.vector.tensor_scalar(out=ac, in0=aggn[r][ib],
                                            scalar1=cond[:, ib:ib + 1],
                                            scalar2=None, op0=ALU.mult)
                    for kc in range(OC):
                        pt = psT.tile([128, 128], dt.bfloat16, tag="tp", name="tp")
                        nc.tensor.transpose(pt, ac[:, kc * 128:(kc + 1) * 128],
                                            ident)
                        nc.scalar.copy(aggTin[r][kc][:, ib * 128:(ib + 1) * 128], pt)
            # aggT = sum_r Bbd_r-rotated aggregate, then gelu -> gT
            gT = []
            for ot in range(OC):
                pb = psB.tile([128, I], dt.float32, tag="aggTp", name="aggTp")
                for r in range(R):
                    for kc in range(OC):
                        nc.tensor.matmul(pb,
                                         bbd_t[r][kc][:, ot * 128:(ot + 1) * 128],
                                         aggTin[r][kc],
                                         start=(r == 0 and kc == 0),
                                         stop=(r == R - 1 and kc == OC - 1))
                g = const.tile([128, I], dt.bfloat16, tag=f"gT{ot}", name=f"gT{ot}")
                nc.scalar.activation(g, pb, AF.Gelu)
                gT.append(g)
            # per-type head: trans = g @ Wa_t + ba_t ; blend ; LN ; type mix
            for ib in range(NIB):
                o_prev = None
                for t_ in range(T):
                    po = psO.tile([128, OUT], dt.float32, tag="trp", name="trp")
                    for kc in range(OC):
                        nc.tensor.matmul(po, gT[kc][:, ib * 128:(ib + 1) * 128],
                                         wa_t[t_][kc], start=(kc == 0), stop=False)
                    nc.tensor.matmul(po, onesRow[0:1, ib * 128:(ib + 1) * 128],
                                     wa_t[t_][2], start=False, stop=True)
                    rt = work.tile([128, OUT], dt.bfloat16, tag="rt", name="rt",
                                   bufs=2)
                    nc.vector.scalar_tensor_tensor(out=rt, in0=po,
                                                   scalar=alphB[t_], in1=xa[t_][ib],
                                                   op0=ALU.mult, op1=ALU.add)
                    st6 = work.tile([128, 6], dt.float32, tag="st6", name="st6",
                                    bufs=2)
                    nc.vector.bn_stats(out=st6, in_=rt)
                    mv = work.tile([128, 2], dt.float32, tag="mv", name="mv",
                                   bufs=2)
                    nc.vector.bn_aggr(out=mv, in_=st6)
                    ve = work.tile([128, 1], dt.float32, tag="ve", name="ve",
                                   bufs=2)
                    nc.vector.tensor_scalar(out=ve, in0=mv[:, 1:2], scalar1=LN_EPS,
                                            scalar2=None, op0=ALU.add)
                    sd = work.tile([128, 1], dt.float32, tag="sd", name="sd",
                                   bufs=2)
                    nc.scalar.sqrt(sd, ve)
                    rstd = work.tile([128, 1], dt.float32, tag="rstd", name="rstd",
                                     bufs=2)
                    nc.vector.reciprocal(rstd, sd)
                    cen = work.tile([128, OUT], dt.bfloat16, tag="cen", name="cen",
                                    bufs=2)
                    nc.vector.tensor_scalar(out=cen, in0=rt, scalar1=mv[:, 0:1],
                                            scalar2=None, op0=ALU.subtract)
                    v1 = work.tile([128, OUT], dt.bfloat16, tag="v1", name="v1",
                                   bufs=2)
                    nc.vector.scalar_tensor_tensor(out=v1, in0=cen, scalar=rstd,
                                                   in1=gB[t_], op0=ALU.mult,
                                                   op1=ALU.mult)
                    tycol = tyI_t[ib][:, t_:t_ + 1]
                    if t_ == 0:
                        ob = work.tile([128, OUT], dt.bfloat16, tag="ob0",
                                       name="ob0", bufs=2)
                        nc.vector.tensor_scalar(out=ob, in0=bB[t_], scalar1=tycol,
                                                scalar2=None, op0=ALU.mult)
                    else:
                        ob = work.tile([128, OUT], dt.bfloat16, tag=f"ob{t_}",
                                       name=f"ob{t_}", bufs=2)
                        nc.vector.scalar_tensor_tensor(out=ob, in0=bB[t_],
                                                       scalar=tycol, in1=o_prev,
                                                       op0=ALU.mult, op1=ALU.add)
                    odt = dt.float32 if t_ == T - 1 else dt.bfloat16
                    onew = work.tile([128, OUT], odt, tag=f"oacc{t_}",
                                     name=f"oacc{t_}", bufs=2)
                    nc.vector.scalar_tensor_tensor(out=onew, in0=v1, scalar=tycol,
                                                   in1=ob, op0=ALU.mult, op1=ALU.add)
                    o_prev = onew
                nc.sync.dma_start(out=outP[ib * 128:(ib + 1) * 128, :], in_=o_prev)

    nc.compile()
    _built["nc"] = nc
    return nc


def _host_prep(inputs):
    """Build the 8 per-core input dicts (numpy, host-side sharding/casts)."""
    f32 = np.float32
    x_all = np.asarray(inputs["node_features"], f32)
    ty_all = np.asarray(inputs["node_types_soft"], f32)
    adj_all = np.asarray(inputs["adj_matrix_soft"], f32)
    erel_all = np.asarray(inputs["edge_types_soft"], f32)
    Wq = np.asarray(inputs["Wq"], f32); bq = np.asarray(inputs["bq"], f32)
    Wk = np.asarray(inputs["Wk"], f32); bk = np.asarray(inputs["bk"], f32)
    Wv = np.asarray(inputs["Wv"], f32); bv = np.asarray(inputs["bv"], f32)
    Wa = np.asarray(inputs["Wa"], f32); ba = np.asarray(inputs["ba"], f32)
    rel_pri = np.asarray(inputs["rel_pri"], f32)
    rel_att = np.asarray(inputs["rel_att"], f32)
    rel_msg = np.asarray(inputs["rel_msg"], f32)
    skip = np.asarray(inputs["skip"], f32)
    lng = np.asarray(inputs["ln_gamma"], f32)
    lnb = np.asarray(inputs["ln_beta"], f32)

    sqrt_dk = math.sqrt(DK)
    abd = np.zeros((R, OUT, OUT), f32)
    bbd = np.zeros((R, OUT, OUT), f32)
    for r in range(R):
        for h in range(H):
            sl = slice(h * DK, (h + 1) * DK)
            abd[r, sl, sl] = rel_att[r, h].T * (rel_pri[r, h] / sqrt_dk)
            bbd[r, sl, sl] = rel_msg[r, h]
    alpha = 1.0 / (1.0 + np.exp(-skip))
    alph = np.stack([alpha, 1.0 - alpha]).astype(f32)

    def bf(a):
        return np.ascontiguousarray(a.astype(BF16))

    wq_aug = bf(np.concatenate([Wq, bq[:, None, :]], axis=1))
    wk_aug = bf(np.concatenate([Wk, bk[:, None, :]], axis=1))
    wv_aug = bf(np.concatenate([Wv, bv[:, None, :]], axis=1))
    wa_aug = bf(np.concatenate([Wa, ba[:, None, :]], axis=1))
    abd_b, bbd_b = bf(abd), bf(bbd)
    lng_b, lnb_b = bf(lng), bf(lnb)

    in_maps = []
    for c in range(NCORES):
        b, half = c // 2, c % 2
        isl = slice(half * I, half * I + I)
        x = x_all[b]
        ty = ty_all[b]
        xT_aug = np.concatenate([x.T, np.ones((1, N), f32)], axis=0)
        xTs_aug = np.stack([
            np.concatenate([(x * ty[:, s:s + 1]).T, ty[None, :, s]], axis=0)
            for s in range(T)])
        in_maps.append({
            "xT": bf(xT_aug),
            "xTI": bf(xT_aug[:, isl]),
            "xI": bf(x[isl]),
            "xTs": bf(xTs_aug),
            "xTsI": bf(xTs_aug[:, :, isl]),
            "typesI": np.ascontiguousarray(ty[isl]),
            "adjT": bf(adj_all[b][isl, :].T),
            "erelT": bf(erel_all[b][isl, :, :].transpose(2, 1, 0)),
            "wq": wq_aug, "wk": wk_aug, "wv": wv_aug, "wa": wa_aug,
            "abd": abd_b, "bbd": bbd_b, "lng": lng_b, "lnb": lnb_b,
            "alph": alph,
        })
    return in_maps


def kernel(**inputs):
    from concourse.bass_utils import run_bass_kernel_spmd
    nc = _build_nc()
    in_maps = _host_prep(inputs)
    res = None
    for attempt in range(3):
        try:
            res = run_bass_kernel_spmd(nc, in_maps, core_ids=list(range(NCORES)))
            break
        except Exception:
            # transient accelerator/tunnel failures recover on retry
            if attempt == 2:
                raise
            import time
            time.sleep(10)
    out = np.zeros((B, N, OUT), np.float32)
    for c in range(NCORES):
        b, half = c // 2, c % 2
        out[b, half * I:half * I + I, :] = res.results[c]["out"]
    return out
